# revision 16
# baseline (speedup 1.0000x reference)
"""Trainium2 Bass kernel for nn_AutoEncoder_53781580481200 (moe_routing).

Strategy (8-core data-parallel over atoms, image-aligned shards):
  host: image-aligned sharding; per-shard stable sort of atoms by symbol
        (routing); per-(symbol,image) run boundary tables.
  device (per core):
        DMA-transpose loads of bf16 x directly into [D, atoms] layout,
        per-symbol MLP: bf16 L1 matmul, float32r L2/L3 (full PE rate),
        ReLU+bias split across ACT (L1) and DVE (L2); energies accumulate
        as PSUM columns (e[m,c] = atom c*128+m) in one pinned bank -> a
        single evacuation + one triangular matmul gives within-column
        prefixes (pp) and a 288-wide scan gives column prefixes (cp).
  host: gp(q) = cp[q//128 - 1] + pp[q%128, q//128]; per-image energies =
        prefix diffs at run boundaries + per-symbol affine constants x
        run counts (O(B) work).
"""

import numpy as np
import ml_dtypes

import concourse.bass as bass
import concourse.bacc as bacc
import concourse.mybir as mybir
import concourse.tile as tile
from concourse.bass_utils import run_bass_kernel_spmd

# problem constants
N, D, H, S, B = 262144, 128, 128, 4, 1024
NCORES = 8

# kernel tiling constants
NG = 8704            # padded atoms per (core, symbol) group: 17 tiles of 512
NS = S * NG          # padded atoms per core = 34816 = 68 tiles
BL = 256             # image slots per core (real <= ~140)
T = 512              # atoms per compute tile
CHUNK = 2048         # atoms per load chunk (1 MB)
SUPER = 4096         # atoms per e-writeback strip

F32 = mybir.dt.float32
F32R = mybir.dt.float32r
I32 = mybir.dt.int32
BF16 = mybir.dt.bfloat16
AF = mybir.ActivationFunctionType
ALU = mybir.AluOpType


def build_nc():
    nc = bacc.Bacc()

    xs_d = nc.declare_dram_parameter("xs", [NS, D], BF16, isOutput=False)
    w1_d = nc.declare_dram_parameter("W1", [S, D, H], F32, isOutput=False)
    w2_d = nc.declare_dram_parameter("W2", [S, H, H], F32, isOutput=False)
    w3_d = nc.declare_dram_parameter("W3c", [S, H], F32, isOutput=False)
    b1_d = nc.declare_dram_parameter("b1", [S, H], F32, isOutput=False)
    b2_d = nc.declare_dram_parameter("b2", [S, H], F32, isOutput=False)
    tri_d = nc.declare_dram_parameter("tri", [128, 128], F32, isOutput=False)
    pp_d = nc.declare_dram_parameter("pp", [128, NS // 128], F32, isOutput=True)
    cp_d = nc.declare_dram_parameter("cp", [1, NS // 128], F32, isOutput=True)

    KC = NS // 128  # 288

    with tile.TileContext(nc) as tc:
        with (
            tc.tile_pool(name="const", bufs=1) as cpool,
            tc.tile_pool(name="xload", bufs=3) as gpool,
            tc.tile_pool(name="h1", bufs=2) as h1pool,
            tc.tile_pool(name="h2", bufs=2) as h2pool,
            tc.tile_pool(name="seg", bufs=1) as spool,
            tc.tile_pool(name="ph1", bufs=2, space="PSUM") as ph1,
            tc.tile_pool(name="ph2", bufs=2, space="PSUM") as ph2,
            tc.tile_pool(name="pea", bufs=1, space="PSUM") as pea,
        ):
            # ---- preload constants ----
            tri_sb = cpool.tile([128, 128], F32, tag="tri")
            nc.sync.dma_start(out=tri_sb[:], in_=tri_d[:])

            w1_sb, w2_sb, w3_sb, b1_sb, b2_sb = [], [], [], [], []
            for s in range(S):
                w1t = cpool.tile([128, 128], F32, tag=f"w1s_{s}")
                nc.sync.dma_start(out=w1t[:], in_=w1_d[s])
                w1r = cpool.tile([128, 128], BF16, tag=f"w1_{s}")
                nc.vector.tensor_copy(out=w1r[:], in_=w1t[:])
                w1_sb.append(w1r)
                w2t = cpool.tile([128, 128], F32, tag=f"w2s_{s}")
                nc.sync.dma_start(out=w2t[:], in_=w2_d[s])
                w2r = cpool.tile([128, 128], F32R, tag=f"w2_{s}")
                nc.vector.tensor_copy(out=w2r[:], in_=w2t[:])
                w2_sb.append(w2r)
                w3t = cpool.tile([128, 1], F32, tag=f"w3s_{s}")
                nc.sync.dma_start(
                    out=w3t[:], in_=w3_d[s].rearrange("(h o) -> h o", o=1)
                )
                w3_sb.append(w3t)
                b1t = cpool.tile([128, 1], F32, tag=f"b1_{s}")
                nc.sync.dma_start(
                    out=b1t[:], in_=b1_d[s].rearrange("(h o) -> h o", o=1)
                )
                b1_sb.append(b1t)
                b2t = cpool.tile([128, 1], F32, tag=f"b2_{s}")
                nc.sync.dma_start(
                    out=b2t[:], in_=b2_d[s].rearrange("(h o) -> h o", o=1)
                )
                b2_sb.append(b2t)

            # ---- main MoE pipeline ----
            # DMA-transpose loads: xt_chunk[:, a] = xs[ch*CHUNK + a, :]
            # e accumulates as PSUM columns: e_all[m, c] = energy of stream
            # atom c*128 + m (one pinned bank for the whole core)
            e_all = pea.tile([128, KC], F32, tag="eall")
            for ch in range(NS // CHUNK):
                xt_chunk = gpool.tile([128, CHUNK], BF16, tag="xtc")
                nc.sync.dma_start(
                    out=xt_chunk[:],
                    in_=xs_d[ch * CHUNK : (ch + 1) * CHUNK, :],
                    transpose=True,
                )
                for t in range(CHUNK // T):  # 4 tiles of 512 atoms
                    tt = ch * (CHUNK // T) + t  # global tile id
                    s = tt // (NG // T)  # symbol of this tile
                    h1_ps = ph1.tile([128, T], F32, tag="h1_ps")
                    nc.tensor.matmul(
                        out=h1_ps[:], lhsT=w1_sb[s][:],
                        rhs=xt_chunk[:, t * T : (t + 1) * T],
                        start=True, stop=True,
                    )
                    h1_sb = h1pool.tile([128, T], F32R, tag="h1_sb")
                    nc.scalar.activation(
                        out=h1_sb[:], in_=h1_ps[:], func=AF.Relu,
                        bias=b1_sb[s][:, 0:1],
                    )

                    h2_ps = ph2.tile([128, T], F32, tag="h2_ps")
                    nc.tensor.matmul(
                        out=h2_ps[:], lhsT=w2_sb[s][:], rhs=h1_sb[:],
                        start=True, stop=True,
                    )
                    h2_sb = h2pool.tile([128, T], F32R, tag="h2_sb")
                    nc.vector.tensor_scalar(
                        out=h2_sb[:], in0=h2_ps[:],
                        scalar1=b2_sb[s][:, 0:1], scalar2=0.0,
                        op0=ALU.add, op1=ALU.max,
                    )

                    for j in range(T // 128):  # e columns, 128 atoms each
                        c = tt * (T // 128) + j
                        nc.tensor.matmul(
                            out=e_all[:, c : c + 1],
                            lhsT=h2_sb[:, j * 128 : (j + 1) * 128].bitcast(F32),
                            rhs=w3_sb[s][:, 0:1],
                            start=True, stop=True,
                        )

            # ---- prefix structure for the host-side boundary diffs ----
            e_sb = spool.tile([128, KC], F32, tag="e_sb")
            nc.vector.tensor_copy(out=e_sb[:], in_=e_all[:])
            # pp[m, c] = sum_{m'<=m} e[c*128+m']  (within-column prefix)
            pp_ps = pea.tile([128, KC], F32, tag="pp")
            nc.tensor.matmul(
                out=pp_ps[:], lhsT=tri_sb[:], rhs=e_sb[:],
                start=True, stop=True,
            )
            pp_sb = spool.tile([128, KC], F32, tag="pp_sb")
            nc.vector.tensor_copy(out=pp_sb[:], in_=pp_ps[:])
            nc.sync.dma_start(out=pp_d[:], in_=pp_sb[:])
            # cp[c] = inclusive prefix of column sums; tri[:,127] is all-ones
            cs_ps = ph1.tile([1, KC], F32, tag="h1_ps")
            nc.tensor.matmul(
                out=cs_ps[:], lhsT=tri_sb[:, 127:128], rhs=e_sb[:],
                start=True, stop=True,
            )
            cs_sb = spool.tile([1, KC], F32, tag="cs_sb")
            nc.vector.tensor_copy(out=cs_sb[:], in_=cs_ps[:])
            zeros1 = spool.tile([1, KC], F32, tag="zeros1")
            nc.vector.memset(zeros1[:], 0.0)
            cp_sb = spool.tile([1, KC], F32, tag="cp_sb")
            nc.vector.tensor_tensor_scan(
                out=cp_sb[:], data0=cs_sb[:], data1=zeros1[:],
                initial=0.0, op0=ALU.add, op1=ALU.add,
            )
            nc.sync.dma_start(out=cp_d[:], in_=cp_sb[:])
    nc.finalize()
    return nc


def prepare_inputs(x, symbol_ids, image_ids, W1, b1, W2, b2, W3, b3, slope,
                   intercept):
    """Image-aligned shards; symbol-grouped atom permutation applied on host;
    run boundary tables kept host-side. Returns (in_maps, metas)."""
    x = np.ascontiguousarray(np.asarray(x, dtype=np.float32))
    sym = np.asarray(symbol_ids, dtype=np.int32)
    img = np.asarray(image_ids, dtype=np.int32)
    W1 = np.ascontiguousarray(np.asarray(W1, np.float32))
    W2 = np.ascontiguousarray(np.asarray(W2, np.float32))
    W3 = np.asarray(W3, np.float32)
    b1 = np.ascontiguousarray(np.asarray(b1, np.float32))
    b2 = np.ascontiguousarray(np.asarray(b2, np.float32))
    b3 = np.asarray(b3, np.float32)
    slope = np.asarray(slope, np.float32)
    intercept = np.asarray(intercept, np.float32)

    W3c = np.ascontiguousarray(W3 * slope[:, None]).astype(np.float32)
    cvec = (slope * b3 + intercept).astype(np.float32).reshape(1, S)
    tri = np.triu(np.ones((128, 128), np.float32), 0)

    cuts = [0]
    for k in range(1, NCORES):
        pos = k * N // NCORES
        cuts.append(int(np.searchsorted(img, img[pos], "left")))
    cuts.append(N)

    in_maps, metas = [], []
    for k in range(NCORES):
        lo, hi = cuts[k], cuts[k + 1]
        ssh = sym[lo:hi]
        ish = img[lo:hi]
        img_lo = int(ish[0])
        nimg = int(ish[-1]) + 1 - img_lo
        assert nimg <= BL, nimg

        order = np.argsort(ssh, kind="stable").astype(np.int64)
        gsyms = ssh[order]
        xsrc = x[lo:hi]
        xs = np.zeros((NS, D), ml_dtypes.bfloat16)
        bnd = np.zeros(S * (BL + 1), np.int64)
        cnts = np.zeros((S, BL), np.int64)
        for s in range(S):
            gl = int(np.searchsorted(gsyms, s, "left"))
            gr = int(np.searchsorted(gsyms, s, "right"))
            cnt = gr - gl
            assert cnt <= NG, cnt
            gidx = order[gl:gr]
            base = s * NG
            xs[base : base + cnt] = xsrc[gidx]
            gimg = ish[gidx]
            ends = np.searchsorted(gimg, np.arange(img_lo, img_lo + BL), "right")
            bnd[s * (BL + 1) : s * (BL + 1) + BL] = base + ends - 1
            bnd[s * (BL + 1) + BL] = base + NG - 1
            cnts[s] = np.diff(np.concatenate([[0], ends]))
        in_maps.append(
            dict(xs=xs, W1=W1, W2=W2, W3c=W3c, b1=b1, b2=b2, tri=tri)
        )
        metas.append((img_lo, nimg, bnd, cnts, cvec))
    return in_maps, metas


def finish_output(results, metas):
    """Per-image energies from device prefix sums: O(B) boundary diffs."""
    out = np.zeros(B, np.float32)
    for k in range(NCORES):
        img_lo, nimg, bnd, cnts, cvec = metas[k]
        pp = np.asarray(results[k]["pp"], np.float64)
        cp = np.asarray(results[k]["cp"], np.float64).ravel()
        cpx = np.concatenate([[0.0], cp[:-1]])  # exclusive column prefix
        q = bnd
        gpv = np.where(q >= 0, cpx[q // 128] + pp[q % 128, q // 128], 0.0)
        t = np.concatenate([[0.0], gpv])
        rs = (t[1:] - t[:-1]).reshape(S, BL + 1)[:, :BL]
        rs = rs + cvec.reshape(S, 1) * cnts  # per-symbol affine constants
        out[img_lo : img_lo + nimg] = rs.sum(axis=0)[:nimg]
    return out


_NC_CACHE = None


def kernel(**inputs):
    global _NC_CACHE
    in_maps, metas = prepare_inputs(**inputs)
    if _NC_CACHE is None:
        _NC_CACHE = build_nc()
    res = run_bass_kernel_spmd(_NC_CACHE, in_maps, list(range(NCORES))).results
    return finish_output(res, metas)


# revision 17
# speedup vs baseline: 1.3439x; 1.3439x over previous
"""Trainium2 Bass kernel for nn_AutoEncoder_53781580481200 (moe_routing).

Strategy (8-core data-parallel over atoms, image-aligned shards):
  host: image-aligned sharding; per-shard stable sort of atoms by symbol
        (routing); per-(symbol,image) run boundary tables.
  device (per core):
        DMA-transpose loads of bf16 x directly into [D, atoms] layout,
        per-symbol MLP: bf16 L1 matmul, float32r L2/L3 (full PE rate),
        ReLU+bias split across ACT (L1) and DVE (L2); energies accumulate
        as PSUM columns (e[m,c] = atom c*128+m) in one pinned bank -> a
        single evacuation + one triangular matmul gives within-column
        prefixes (pp) and a 288-wide scan gives column prefixes (cp).
  host: gp(q) = cp[q//128 - 1] + pp[q%128, q//128]; per-image energies =
        prefix diffs at run boundaries + per-symbol affine constants x
        run counts (O(B) work).
"""

import numpy as np
import ml_dtypes

import concourse.bass as bass
import concourse.bacc as bacc
import concourse.mybir as mybir
import concourse.tile as tile
from concourse.bass_utils import run_bass_kernel_spmd

# problem constants
N, D, H, S, B = 262144, 128, 128, 4, 1024
NCORES = 8

# kernel tiling constants
NG = 8704            # padded atoms per (core, symbol) group: 17 tiles of 512
NS = S * NG          # padded atoms per core = 34816 = 68 tiles
BL = 256             # image slots per core (real <= ~140)
T = 512              # atoms per compute tile
CHUNK = 2048         # atoms per load chunk (1 MB)
SUPER = 4096         # atoms per e-writeback strip

F32 = mybir.dt.float32
F32R = mybir.dt.float32r
I32 = mybir.dt.int32
BF16 = mybir.dt.bfloat16
AF = mybir.ActivationFunctionType
ALU = mybir.AluOpType


def build_nc():
    nc = bacc.Bacc()

    xs_d = nc.declare_dram_parameter("xs", [NS, D], BF16, isOutput=False)
    w1_d = nc.declare_dram_parameter("W1", [S, D, H], F32, isOutput=False)
    w2_d = nc.declare_dram_parameter("W2", [S, H, H], F32, isOutput=False)
    w3_d = nc.declare_dram_parameter("W3c", [S, H], F32, isOutput=False)
    b1_d = nc.declare_dram_parameter("b1", [S, H], F32, isOutput=False)
    b2_d = nc.declare_dram_parameter("b2", [S, H], F32, isOutput=False)
    tri_d = nc.declare_dram_parameter("tri", [128, 128], F32, isOutput=False)
    pp_d = nc.declare_dram_parameter("pp", [128, NS // 128], F32, isOutput=True)
    cp_d = nc.declare_dram_parameter("cp", [1, NS // 128], F32, isOutput=True)

    KC = NS // 128  # 288

    with tile.TileContext(nc) as tc:
        with (
            tc.tile_pool(name="const", bufs=1) as cpool,
            tc.tile_pool(name="xload", bufs=4) as gpool,
            tc.tile_pool(name="h1", bufs=3) as h1pool,
            tc.tile_pool(name="h2", bufs=3) as h2pool,
            tc.tile_pool(name="seg", bufs=1) as spool,
            tc.tile_pool(name="ph1", bufs=3, space="PSUM") as ph1,
            tc.tile_pool(name="ph2", bufs=3, space="PSUM") as ph2,
            tc.tile_pool(name="pea", bufs=1, space="PSUM") as pea,
        ):
            # ---- preload constants ----
            tri_sb = cpool.tile([128, 128], F32, tag="tri")
            nc.sync.dma_start(out=tri_sb[:], in_=tri_d[:])

            w1_sb, w2_sb, w3_sb, b1_sb, b2_sb = [], [], [], [], []
            for s in range(S):
                w1t = cpool.tile([128, 128], F32, tag=f"w1s_{s}")
                nc.sync.dma_start(out=w1t[:], in_=w1_d[s])
                w1r = cpool.tile([128, 128], BF16, tag=f"w1_{s}")
                nc.vector.tensor_copy(out=w1r[:], in_=w1t[:])
                w1_sb.append(w1r)
                w2t = cpool.tile([128, 128], F32, tag=f"w2s_{s}")
                nc.sync.dma_start(out=w2t[:], in_=w2_d[s])
                w2r = cpool.tile([128, 128], F32R, tag=f"w2_{s}")
                nc.vector.tensor_copy(out=w2r[:], in_=w2t[:])
                w2_sb.append(w2r)
                w3t = cpool.tile([128, 1], F32, tag=f"w3s_{s}")
                nc.sync.dma_start(
                    out=w3t[:], in_=w3_d[s].rearrange("(h o) -> h o", o=1)
                )
                w3_sb.append(w3t)
                b1t = cpool.tile([128, 1], F32, tag=f"b1_{s}")
                nc.sync.dma_start(
                    out=b1t[:], in_=b1_d[s].rearrange("(h o) -> h o", o=1)
                )
                b1_sb.append(b1t)
                b2t = cpool.tile([128, 1], F32, tag=f"b2_{s}")
                nc.sync.dma_start(
                    out=b2t[:], in_=b2_d[s].rearrange("(h o) -> h o", o=1)
                )
                b2_sb.append(b2t)

            # ---- main MoE pipeline ----
            # DMA-transpose loads: xt_chunk[:, a] = xs[ch*CHUNK + a, :]
            # e accumulates as PSUM columns: e_all[m, c] = energy of stream
            # atom c*128 + m (one pinned bank for the whole core)
            e_all = pea.tile([128, KC], F32, tag="eall")
            for ch in range(NS // CHUNK):
                xt_chunk = gpool.tile([128, CHUNK], BF16, tag="xtc")
                nc.sync.dma_start(
                    out=xt_chunk[:],
                    in_=xs_d[ch * CHUNK : (ch + 1) * CHUNK, :],
                    transpose=True,
                )
                for t in range(CHUNK // T):  # 4 tiles of 512 atoms
                    tt = ch * (CHUNK // T) + t  # global tile id
                    s = tt // (NG // T)  # symbol of this tile
                    h1_ps = ph1.tile([128, T], F32, tag="h1_ps")
                    nc.tensor.matmul(
                        out=h1_ps[:], lhsT=w1_sb[s][:],
                        rhs=xt_chunk[:, t * T : (t + 1) * T],
                        start=True, stop=True,
                    )
                    h1_sb = h1pool.tile([128, T], F32R, tag="h1_sb")
                    nc.scalar.activation(
                        out=h1_sb[:], in_=h1_ps[:], func=AF.Relu,
                        bias=b1_sb[s][:, 0:1],
                    )

                    h2_ps = ph2.tile([128, T], F32, tag="h2_ps")
                    nc.tensor.matmul(
                        out=h2_ps[:], lhsT=w2_sb[s][:], rhs=h1_sb[:],
                        start=True, stop=True,
                    )
                    h2_sb = h2pool.tile([128, T], F32R, tag="h2_sb")
                    nc.vector.tensor_scalar(
                        out=h2_sb[:], in0=h2_ps[:],
                        scalar1=b2_sb[s][:, 0:1], scalar2=0.0,
                        op0=ALU.add, op1=ALU.max,
                    )

                    for j in range(T // 128):  # e columns, 128 atoms each
                        c = tt * (T // 128) + j
                        nc.tensor.matmul(
                            out=e_all[:, c : c + 1],
                            lhsT=h2_sb[:, j * 128 : (j + 1) * 128].bitcast(F32),
                            rhs=w3_sb[s][:, 0:1],
                            start=True, stop=True,
                        )

            # ---- prefix structure for the host-side boundary diffs ----
            e_sb = spool.tile([128, KC], F32, tag="e_sb")
            nc.vector.tensor_copy(out=e_sb[:], in_=e_all[:])
            # pp[m, c] = sum_{m'<=m} e[c*128+m']  (within-column prefix)
            pp_ps = pea.tile([128, KC], F32, tag="pp")
            nc.tensor.matmul(
                out=pp_ps[:], lhsT=tri_sb[:], rhs=e_sb[:],
                start=True, stop=True,
            )
            pp_sb = spool.tile([128, KC], F32, tag="pp_sb")
            nc.vector.tensor_copy(out=pp_sb[:], in_=pp_ps[:])
            nc.sync.dma_start(out=pp_d[:], in_=pp_sb[:])
            # cp[c] = inclusive prefix of column sums; tri[:,127] is all-ones
            cs_ps = ph1.tile([1, KC], F32, tag="h1_ps")
            nc.tensor.matmul(
                out=cs_ps[:], lhsT=tri_sb[:, 127:128], rhs=e_sb[:],
                start=True, stop=True,
            )
            cs_sb = spool.tile([1, KC], F32, tag="cs_sb")
            nc.vector.tensor_copy(out=cs_sb[:], in_=cs_ps[:])
            zeros1 = spool.tile([1, KC], F32, tag="zeros1")
            nc.vector.memset(zeros1[:], 0.0)
            cp_sb = spool.tile([1, KC], F32, tag="cp_sb")
            nc.vector.tensor_tensor_scan(
                out=cp_sb[:], data0=cs_sb[:], data1=zeros1[:],
                initial=0.0, op0=ALU.add, op1=ALU.add,
            )
            nc.sync.dma_start(out=cp_d[:], in_=cp_sb[:])
    nc.finalize()
    return nc


def prepare_inputs(x, symbol_ids, image_ids, W1, b1, W2, b2, W3, b3, slope,
                   intercept):
    """Image-aligned shards; symbol-grouped atom permutation applied on host;
    run boundary tables kept host-side. Returns (in_maps, metas)."""
    x = np.ascontiguousarray(np.asarray(x, dtype=np.float32))
    sym = np.asarray(symbol_ids, dtype=np.int32)
    img = np.asarray(image_ids, dtype=np.int32)
    W1 = np.ascontiguousarray(np.asarray(W1, np.float32))
    W2 = np.ascontiguousarray(np.asarray(W2, np.float32))
    W3 = np.asarray(W3, np.float32)
    b1 = np.ascontiguousarray(np.asarray(b1, np.float32))
    b2 = np.ascontiguousarray(np.asarray(b2, np.float32))
    b3 = np.asarray(b3, np.float32)
    slope = np.asarray(slope, np.float32)
    intercept = np.asarray(intercept, np.float32)

    W3c = np.ascontiguousarray(W3 * slope[:, None]).astype(np.float32)
    cvec = (slope * b3 + intercept).astype(np.float32).reshape(1, S)
    tri = np.triu(np.ones((128, 128), np.float32), 0)

    cuts = [0]
    for k in range(1, NCORES):
        pos = k * N // NCORES
        cuts.append(int(np.searchsorted(img, img[pos], "left")))
    cuts.append(N)

    in_maps, metas = [], []
    for k in range(NCORES):
        lo, hi = cuts[k], cuts[k + 1]
        ssh = sym[lo:hi]
        ish = img[lo:hi]
        img_lo = int(ish[0])
        nimg = int(ish[-1]) + 1 - img_lo
        assert nimg <= BL, nimg

        order = np.argsort(ssh, kind="stable").astype(np.int64)
        gsyms = ssh[order]
        xsrc = x[lo:hi]
        xs = np.zeros((NS, D), ml_dtypes.bfloat16)
        bnd = np.zeros(S * (BL + 1), np.int64)
        cnts = np.zeros((S, BL), np.int64)
        for s in range(S):
            gl = int(np.searchsorted(gsyms, s, "left"))
            gr = int(np.searchsorted(gsyms, s, "right"))
            cnt = gr - gl
            assert cnt <= NG, cnt
            gidx = order[gl:gr]
            base = s * NG
            xs[base : base + cnt] = xsrc[gidx]
            gimg = ish[gidx]
            ends = np.searchsorted(gimg, np.arange(img_lo, img_lo + BL), "right")
            bnd[s * (BL + 1) : s * (BL + 1) + BL] = base + ends - 1
            bnd[s * (BL + 1) + BL] = base + NG - 1
            cnts[s] = np.diff(np.concatenate([[0], ends]))
        in_maps.append(
            dict(xs=xs, W1=W1, W2=W2, W3c=W3c, b1=b1, b2=b2, tri=tri)
        )
        metas.append((img_lo, nimg, bnd, cnts, cvec))
    return in_maps, metas


def finish_output(results, metas):
    """Per-image energies from device prefix sums: O(B) boundary diffs."""
    out = np.zeros(B, np.float32)
    for k in range(NCORES):
        img_lo, nimg, bnd, cnts, cvec = metas[k]
        pp = np.asarray(results[k]["pp"], np.float64)
        cp = np.asarray(results[k]["cp"], np.float64).ravel()
        cpx = np.concatenate([[0.0], cp[:-1]])  # exclusive column prefix
        q = bnd
        gpv = np.where(q >= 0, cpx[q // 128] + pp[q % 128, q // 128], 0.0)
        t = np.concatenate([[0.0], gpv])
        rs = (t[1:] - t[:-1]).reshape(S, BL + 1)[:, :BL]
        rs = rs + cvec.reshape(S, 1) * cnts  # per-symbol affine constants
        out[img_lo : img_lo + nimg] = rs.sum(axis=0)[:nimg]
    return out


_NC_CACHE = None


def kernel(**inputs):
    global _NC_CACHE
    in_maps, metas = prepare_inputs(**inputs)
    if _NC_CACHE is None:
        _NC_CACHE = build_nc()
    res = run_bass_kernel_spmd(_NC_CACHE, in_maps, list(range(NCORES))).results
    return finish_output(res, metas)
